# revision 20
# baseline (speedup 1.0000x reference)
"""Trainium2 Bass kernel for nn_Attention_49907519979595 (Bahdanau-style attention).

Math (per batch b):
    q      = query @ Wq.T + bq                      [H]
    r_s    = Wr @ ref_s + br                        [S, H]
    logit  = V . tanh(q + r_s)                      [S]
    w      = softmax(logit)                         [S]
    expected = sum_s w_s r_s = Wr @ (sum_s w_s ref_s) + br   (softmax weights sum to 1)
    result = concat(output, expected) @ Wo.T + bo   [H]

Implementation notes (v2, from trace analysis of the 224us v1):
  - Data-parallel over batch: 8 cores x 8 batches; each core streams its 32 MiB
    ref slice once (memory regime).  DMA loads 4 consecutive s-rows per
    descriptor (4 KiB reads) as bf16 into nat[p, t, c, h]; the s-order inside a
    tile becomes (c,p)-interleaved, which is harmless: softmax + weighted sum
    are order-invariant in s, and every consumer derives its s-order from the
    same transposed tiles.
  - TensorE transposes (bf16 identity) produce refT [h, s] tiles in PSUM;
    one VectorE copy moves each to SBUF.  r = WrT.T @ refT on PE (bf16).
  - tanh(r + (q+br)) on ScalarE via the per-partition bias port.
  - Logit matmuls (V stationary) write row t of a per-batch [8, 512] PSUM
    bank, so ONE exp instruction covers the whole batch (v1 spent 43us in
    64 single-partition exps); exp's accum_out yields the per-tile partial
    Z sums for free.  No-max softmax: |logit| <= ||V||_1 <= 16, safe in f32.
  - Weighted sum: GpSimd broadcasts each e row; ONE fused
    tensor_tensor_reduce per (tile, half) does mul+reduce (v1 used separate
    mul and reduce instructions), accumulating f32 per-tile columns.
  - Software-pipelined emission over 64 tiles keeps PE/ACT/DVE/Pool/DMA
    overlapped; per-(batch,half) accumulators live in offset-0 tiles.
  - Final projection folds Wr into Wo: result = output @ WoA.T
    + (acc/Z) @ (WoB@Wr).T + (WoB@br + bo), all f32 on-device.
"""

import os
import sys

import numpy as np

sys.path.insert(0, "/opt/trn_rl_repo")

H = 256
B = 64
S = 4096
N_CORES = 8
B_CORE = B // N_CORES  # 8
S_TILE = 512
N_STILES = S // S_TILE  # 8

_nc_cache = {}


def build_nc():
    import concourse.bacc as bacc
    import concourse.tile as tile
    from concourse import masks, mybir

    f32 = mybir.dt.float32
    bf16 = mybir.dt.bfloat16
    AF = mybir.ActivationFunctionType
    ALU = mybir.AluOpType

    nc = bacc.Bacc("TRN2", debug=False)
    ref = nc.dram_tensor("ref", [B_CORE, S, H], f32, kind="ExternalInput").ap()
    query = nc.dram_tensor("query", [B_CORE, H], f32, kind="ExternalInput").ap()
    out_prev = nc.dram_tensor("out_prev", [B_CORE, H], f32, kind="ExternalInput").ap()
    Wq = nc.dram_tensor("Wq", [H, H], f32, kind="ExternalInput").ap()
    bq = nc.dram_tensor("bq", [H], f32, kind="ExternalInput").ap()
    Wr = nc.dram_tensor("Wr", [H, H], f32, kind="ExternalInput").ap()
    br = nc.dram_tensor("br", [H], f32, kind="ExternalInput").ap()
    Wo = nc.dram_tensor("Wo", [H, 2 * H], f32, kind="ExternalInput").ap()
    bo = nc.dram_tensor("bo", [H], f32, kind="ExternalInput").ap()
    V = nc.dram_tensor("V", [H], f32, kind="ExternalInput").ap()
    result = nc.dram_tensor("result", [B_CORE, H], f32, kind="ExternalOutput").ap()

    with tile.TileContext(nc) as tc:
        with (
            tc.tile_pool(name="const", bufs=1) as const,
            tc.tile_pool(name="natp", bufs=12) as natp,
            tc.tile_pool(name="reftp", bufs=8) as reftp,
            tc.tile_pool(name="tanhp", bufs=4) as tanhp,
            tc.tile_pool(name="small", bufs=6) as small,
            tc.tile_pool(name="accp", bufs=2) as accp,
        ):
            # ---------------- prologue: weights & biases ----------------
            psum_pro_cm = tc.tile_pool(name="psum_pro", bufs=2, space="PSUM")
            psum_s = psum_pro_cm.__enter__()
            ident = const.tile([128, 128], f32, name="ident")
            masks.make_identity(nc, ident[:])
            ident_bf = const.tile([128, 128], bf16, name="ident_bf")
            nc.vector.tensor_copy(ident_bf[:], ident[:])

            def load_col(vec_ap, name):
                t = const.tile([128, 2], f32, name=name)
                nc.sync.dma_start(t[:], vec_ap.rearrange("(c p) -> p c", p=128))
                return t

            bq_col = load_col(bq, "bq_col")
            br_col = load_col(br, "br_col")
            bo_col = load_col(bo, "bo_col")
            V_f32 = load_col(V, "V_f32")
            V_col = const.tile([128, 2], bf16, name="V_col")
            nc.vector.tensor_copy(V_col[:], V_f32[:])

            def load_rows(mat_ap, ncols, name):
                t = const.tile([128, 2, ncols], f32, name=name)
                nc.sync.dma_start(t[:], mat_ap.rearrange("(c p) n -> p c n", p=128))
                return t

            Wq_nat = load_rows(Wq, H, "Wq_nat")
            Wr_nat = load_rows(Wr, H, "Wr_nat")
            Wo_nat = load_rows(Wo, 2 * H, "Wo_nat")

            def transpose_256(nat, out_dtype, name, col0=0):
                t = const.tile([128, 2, H], out_dtype, name=name)
                for c in range(2):
                    for g in range(2):
                        tp = psum_s.tile([128, 256], f32, name=f"{name}_tp", tag="ps")[:, :128]
                        nc.tensor.transpose(
                            tp[:], nat[:, g, col0 + c * 128 : col0 + (c + 1) * 128], ident[:]
                        )
                        nc.scalar.copy(t[:, c, g * 128 : (g + 1) * 128], tp[:])
                return t

            WqT = transpose_256(Wq_nat, f32, "WqT")
            WrT = transpose_256(Wr_nat, bf16, "WrT")

            # bias_sb[c][p, b] = (Wq @ query[b].T)[c*128+p] + bq + br
            query_sb = const.tile([B_CORE, H], f32, name="query_sb")
            nc.sync.dma_start(query_sb[:], query)
            queryT = const.tile([128, 2, B_CORE], f32, name="queryT")
            for c in range(2):
                qt_ps = psum_s.tile([128, B_CORE], f32, name="qt_ps", tag="ps")
                nc.tensor.transpose(
                    qt_ps[:], query_sb[:, c * 128 : (c + 1) * 128], ident[:B_CORE, :B_CORE]
                )
                nc.scalar.copy(queryT[:, c, :], qt_ps[:])
            bias_sb = const.tile([128, 2, B_CORE], f32, name="bias_sb")
            for c in range(2):
                q_ps = psum_s.tile([128, B_CORE], f32, name="q_ps", tag="ps")
                for ck in range(2):
                    nc.tensor.matmul(
                        q_ps[:],
                        WqT[:, ck, c * 128 : (c + 1) * 128],
                        queryT[:, ck, :],
                        start=(ck == 0),
                        stop=(ck == 1),
                    )
                nc.scalar.activation(
                    bias_sb[:, c, :], q_ps[:], AF.Identity, bias=bq_col[:, c : c + 1]
                )
                nc.scalar.activation(
                    bias_sb[:, c, :], bias_sb[:, c, :], AF.Identity,
                    bias=br_col[:, c : c + 1]
                )

            # per-batch outputs of the streaming phase
            accT_all = const.tile([128, 2, B_CORE], f32, name="accT_all")
            accT_bh = [
                const.tile([128, 1], f32, name=f"accT_bh{i}", tag=f"accT_bh{i}")
                for i in range(2 * B_CORE)
            ]
            z_list = [
                const.tile([1, 1], f32, name=f"zacc{b}", tag=f"zacc{b}")
                for b in range(B_CORE)
            ]

            psum_pro_cm.__exit__(None, None, None)
            psum_r_cm = tc.tile_pool(name="psum_r", bufs=3, space="PSUM")
            psum_r = psum_r_cm.__enter__()
            psum_t_cm = tc.tile_pool(name="psum_t", bufs=2, space="PSUM")
            psum_t = psum_t_cm.__enter__()
            psum_lg_cm = tc.tile_pool(name="psum_lg", bufs=2, space="PSUM")
            psum_lg = psum_lg_cm.__enter__()

            # ---------------- main loop (software-pipelined emission) ----------------
            NT = B_CORE * N_STILES  # 64 global tiles
            st = {}  # per-tile pipeline state

            def emit_load(v):
                # one 512-s tile per DMA: finer pipeline granularity (the
                # first tile's transpose starts ~2us in, not after a full
                # 15us batch load).  4 consecutive s-rows per partition-row
                # (4KiB contiguous DRAM reads); the (p,c) s-interleave within
                # the tile is harmless (softmax is order-invariant in s and
                # every consumer derives its s-order from the same refT).
                b, t = divmod(v, N_STILES)
                nat = natp.tile([128, 4, H], bf16, name="nat", tag="nat")
                nc.gpsimd.dma_start(
                    nat[:],
                    ref[b, t * S_TILE : (t + 1) * S_TILE].rearrange(
                        "(p c) h -> p c h", p=128
                    ),
                )
                st[("nat", v)] = nat

            PREFETCH = 10  # tiles of DMA lookahead

            for v0 in range(min(PREFETCH, NT)):
                emit_load(v0)

            def stage_load(v):
                if v + PREFETCH < NT:
                    emit_load(v + PREFETCH)

            def stage_transpose(v):
                nat = st.pop(("nat", v))
                refT_ps = psum_t.tile([128, 2, S_TILE], bf16, name="refT_ps", tag="rtps")
                for hh in range(2):
                    for c in range(4):
                        nc.tensor.transpose(
                            refT_ps[:, hh, c * 128 : (c + 1) * 128],
                            nat[:, c, hh * 128 : (hh + 1) * 128],
                            ident_bf[:],
                        )
                st[("rtps", v)] = refT_ps

            def stage_copy(v):
                refT_ps = st.pop(("rtps", v))
                refT = reftp.tile([128, 2, S_TILE], bf16, name="refT", tag="refT")
                nc.vector.tensor_copy(refT[:], refT_ps[:])
                st[("refT", v)] = refT

            def stage_r(v):
                refT = st[("refT", v)]
                rr = []
                for hh in range(2):
                    r_ps = psum_r.tile([128, S_TILE], f32, name="r_ps", tag="rps")
                    for ck in range(2):
                        nc.tensor.matmul(
                            r_ps[:],
                            WrT[:, ck, hh * 128 : (hh + 1) * 128],
                            refT[:, ck, :],
                            start=(ck == 0),
                            stop=(ck == 1),
                        )
                    rr.append(r_ps)
                st[("rps", v)] = rr

            def stage_tanh(v):
                b, t = divmod(v, N_STILES)
                rr = st.pop(("rps", v))
                tanh_sb = tanhp.tile([128, 2, S_TILE], bf16, name="tanh_sb", tag="tanh")
                for hh in range(2):
                    nc.scalar.activation(
                        tanh_sb[:, hh, :],
                        rr[hh][:],
                        AF.Tanh,
                        bias=bias_sb[:, hh, b : b + 1],
                    )
                st[("tanh", v)] = tanh_sb

            def stage_lg(v):
                b, t = divmod(v, N_STILES)
                tanh_sb = st.pop(("tanh", v))
                lg_ps = psum_lg.tile([1, S_TILE], f32, name="lg_ps", tag="lg_ps")
                for hh in range(2):
                    nc.tensor.matmul(
                        lg_ps[:],
                        V_col[:, hh : hh + 1],
                        tanh_sb[:, hh, :],
                        start=(hh == 0),
                        stop=(hh == 1),
                    )
                st[("lg", v)] = lg_ps

            def stage_exp(v):
                b, t = divmod(v, N_STILES)
                lg_ps = st.pop(("lg", v))
                e_row = small.tile([1, S_TILE], bf16, name="e_row", tag="e_row")
                zt = small.tile([1, 1], f32, name="zt", tag="zt", bufs=12)
                nc.scalar.activation(e_row[:], lg_ps[:], AF.Exp, accum_out=zt[:])
                st[("zt", v)] = zt
                st[("e_row", v)] = e_row

            def stage_bcast(v):
                e_row = st.pop(("e_row", v))
                e_bc = small.tile([128, S_TILE], bf16, name="e_bc", tag="e_bc")
                nc.gpsimd.partition_broadcast(e_bc[:], e_row[:])
                st[("e_bc", v)] = e_bc

            def stage_acc(v):
                b, t = divmod(v, N_STILES)
                e_bc = st.pop(("e_bc", v))
                refT = st.pop(("refT", v))
                if t == 0:
                    st[("redcols", b)] = accp.tile(
                        [128, 2, N_STILES], bf16, name="redcols", tag="redcols"
                    )
                redcols = st[("redcols", b)]
                # NOTE: tensor_tensor_reduce (fused mul+reduce) crashes HW
                # (INTERNAL error, wedges the device) though CoreSim accepts
                # it — use affine_mul_reduce ucode instead.
                for hh in range(2):
                    prod = small.tile([128, S_TILE], bf16, name="prod", tag=f"prod{hh}")
                    with nc.allow_low_precision(
                        reason="per-tile partial sums; error diluted by output projection"
                    ):
                        nc.vector.affine_mul_reduce(
                            prod[:],
                            redcols[:, hh, t : t + 1],
                            refT[:, hh, :],
                            e_bc[:],
                            1.0,
                            0.0,
                        )
                if t == N_STILES - 1:
                    for hh in range(2):
                        nc.vector.reduce_sum(
                            accT_bh[b * 2 + hh][:],
                            redcols[:, hh, :],
                            axis=mybir.AxisListType.X,
                        )
                    st.pop(("redcols", b))
                    # gather the batch's 8 exp-partials and fold to Z (DMA +
                    # one DVE reduce instead of 8 serial DVE adds)
                    zrow8 = small.tile([1, N_STILES], f32, name="zrow8", tag="zrow8")
                    for tt in range(N_STILES):
                        nc.sync.dma_start(
                            zrow8[:, tt : tt + 1], st.pop(("zt", b * N_STILES + tt))[:]
                        )
                    nc.vector.reduce_sum(
                        z_list[b][:], zrow8[:], axis=mybir.AxisListType.X
                    )

            STAGES = [
                (0, stage_load),
                (0, stage_transpose),
                (1, stage_copy),
                (2, stage_r),
                (3, stage_tanh),
                (4, stage_lg),
                (5, stage_exp),
                (6, stage_bcast),
                (7, stage_acc),
            ]
            MAXLAG = max(lag for lag, _ in STAGES)
            for step in range(NT + MAXLAG):
                for lag, fn in STAGES:
                    w = step - lag
                    if 0 <= w < NT:
                        fn(w)

            # ---------------- epilogue ----------------
            psum_lg_cm.__exit__(None, None, None)
            psum_t_cm.__exit__(None, None, None)
            psum_r_cm.__exit__(None, None, None)
            psum_epi_cm = tc.tile_pool(name="psum_epi", bufs=2, space="PSUM")
            psum_s = psum_epi_cm.__enter__()
            WoAT = transpose_256(Wo_nat, f32, "WoAT", col0=0)
            WoBT = transpose_256(Wo_nat, f32, "WoBT", col0=H)
            # MT = (WoB @ Wr).T  rows chunked: [128, 2, 256]
            MT = const.tile([128, 2, H], f32, name="MT")
            for cm in range(2):
                mt_ps = psum_s.tile([128, H], f32, name="mt_ps", tag="ps")
                for ck in range(2):
                    nc.tensor.matmul(
                        mt_ps[:],
                        Wr_nat[:, ck, cm * 128 : (cm + 1) * 128],
                        WoBT[:, ck, :],
                        start=(ck == 0),
                        stop=(ck == 1),
                    )
                nc.scalar.copy(MT[:, cm, :], mt_ps[:])

            # c_col = WoB @ br + bo   [128, 2]
            c_col = const.tile([128, 2], f32, name="c_col")
            for co in range(2):
                c_ps = psum_s.tile([128, 1], f32, name="c_ps", tag="ps")
                for ck in range(2):
                    nc.tensor.matmul(
                        c_ps[:],
                        WoBT[:, ck, co * 128 : (co + 1) * 128],
                        br_col[:, ck : ck + 1],
                        start=(ck == 0),
                        stop=(ck == 1),
                    )
                nc.scalar.activation(
                    c_col[:, co : co + 1], c_ps[:], AF.Identity, bias=bo_col[:, co : co + 1]
                )

            for b in range(B_CORE):
                for hh in range(2):
                    nc.sync.dma_start(
                        accT_all[:, hh, b : b + 1], accT_bh[b * 2 + hh][:]
                    )
            # Z per batch -> [8, 1] and reciprocal
            zrow = small.tile([B_CORE, 1], f32, name="zrow")
            for b in range(B_CORE):
                nc.sync.dma_start(zrow[b : b + 1, :], z_list[b][:])
            rz = small.tile([B_CORE, 1], f32, name="rz")
            nc.vector.reciprocal(rz[:], zrow[:])

            # u = (accT / Z): transpose accT to [b, h], scale rows, transpose back
            acc_bh = small.tile([B_CORE, H], f32, name="acc_bh")
            for c in range(2):
                ab_ps = psum_s.tile([B_CORE, 128], f32, name="ab_ps", tag="ps")
                nc.tensor.transpose(ab_ps[:], accT_all[:, c, :], ident[:])
                nc.vector.tensor_copy(acc_bh[:, c * 128 : (c + 1) * 128], ab_ps[:])
            u_bh = small.tile([B_CORE, H], f32, name="u_bh")
            nc.vector.tensor_scalar_mul(u_bh[:], acc_bh[:], rz[:])
            uT = small.tile([128, 2, B_CORE], f32, name="uT")
            for c in range(2):
                ut_ps = psum_s.tile([128, B_CORE], f32, name="ut_ps", tag="ps")
                nc.tensor.transpose(
                    ut_ps[:], u_bh[:, c * 128 : (c + 1) * 128], ident[:B_CORE, :B_CORE]
                )
                nc.vector.tensor_copy(uT[:, c, :], ut_ps[:])

            # outputT
            outp_sb = small.tile([B_CORE, H], f32, name="outp_sb")
            nc.sync.dma_start(outp_sb[:], out_prev)
            outT = small.tile([128, 2, B_CORE], f32, name="outT")
            for c in range(2):
                ot_ps = psum_s.tile([128, B_CORE], f32, name="ot_ps", tag="ps")
                nc.tensor.transpose(
                    ot_ps[:], outp_sb[:, c * 128 : (c + 1) * 128], ident[:B_CORE, :B_CORE]
                )
                nc.vector.tensor_copy(outT[:, c, :], ot_ps[:])

            # resultT[co] = WoA.T-chunks @ outT + MT-chunks @ uT  (+ c_col bias)
            res_sb = small.tile([B_CORE, H], f32, name="res_sb")
            for co in range(2):
                r2_ps = psum_s.tile([128, B_CORE], f32, name="r2_ps", tag="ps")
                for ck in range(2):
                    nc.tensor.matmul(
                        r2_ps[:],
                        WoAT[:, ck, co * 128 : (co + 1) * 128],
                        outT[:, ck, :],
                        start=(ck == 0),
                        stop=False,
                        skip_group_check=True,
                    )
                for ck in range(2):
                    nc.tensor.matmul(
                        r2_ps[:],
                        MT[:, ck, co * 128 : (co + 1) * 128],
                        uT[:, ck, :],
                        start=False,
                        stop=(ck == 1),
                        skip_group_check=True,
                    )
                resT_sb = small.tile([128, B_CORE], f32, name="resT_sb")
                nc.scalar.activation(
                    resT_sb[:], r2_ps[:], AF.Identity, bias=c_col[:, co : co + 1]
                )
                rb_ps = psum_s.tile([B_CORE, 128], f32, name="rb_ps", tag="ps")
                nc.tensor.transpose(rb_ps[:], resT_sb[:], ident[:])
                nc.vector.tensor_copy(res_sb[:, co * 128 : (co + 1) * 128], rb_ps[:])

            nc.sync.dma_start(result, res_sb[:])
            psum_epi_cm.__exit__(None, None, None)

    nc.compile()
    return nc


def _get_nc():
    if "nc" not in _nc_cache:
        _nc_cache["nc"] = build_nc()
    return _nc_cache["nc"]


def kernel(output, query, ref, Wq, bq, Wr, br, Wo, bo, V):
    from concourse.bass_utils import run_bass_kernel_spmd

    output = np.ascontiguousarray(np.asarray(output, dtype=np.float32))
    query = np.ascontiguousarray(np.asarray(query, dtype=np.float32))
    ref = np.ascontiguousarray(np.asarray(ref, dtype=np.float32))
    shared = {
        "Wq": np.ascontiguousarray(np.asarray(Wq, np.float32)),
        "bq": np.ascontiguousarray(np.asarray(bq, np.float32)),
        "Wr": np.ascontiguousarray(np.asarray(Wr, np.float32)),
        "br": np.ascontiguousarray(np.asarray(br, np.float32)),
        "Wo": np.ascontiguousarray(np.asarray(Wo, np.float32)),
        "bo": np.ascontiguousarray(np.asarray(bo, np.float32)),
        "V": np.ascontiguousarray(np.asarray(V, np.float32)),
    }

    nc = _get_nc()
    in_maps = []
    for c in range(N_CORES):
        sl = slice(c * B_CORE, (c + 1) * B_CORE)
        in_maps.append(
            {
                "ref": ref[sl],
                "query": query[sl],
                "out_prev": output[sl],
                **shared,
            }
        )

    trace = bool(int(os.environ.get("KERNEL_TRACE", "0")))
    res = run_bass_kernel_spmd(nc, in_maps, list(range(N_CORES)), trace=trace)
    if trace:
        kernel.last_exec_time_ns = res.exec_time_ns
        kernel.last_profile = res
    out = np.concatenate([res.results[c]["result"] for c in range(N_CORES)], axis=0)
    return out.reshape(B, 1, H)
